# revision 36
# baseline (speedup 1.0000x reference)
"""Trainium2 Bass kernel for nn_LogicConvSparseMatrix.

Math: the reference's 15-term weighted logic-op sum collapses to

    out[b,k] = C_ab[k]*A*B + C_a[k]*A + C_b[k]*B + C_1[k]

where A = x[b, ca_k, ha_k+oh, wa_k+ow], B = x[b, cb_k, hb_k+oh, wb_k+ow]
are shifted 126x126 windows.  Per kernel, with P = one operand and Q =
the other (orientation chosen per kernel), this factors into

    out = (Q + alpha) * (C_ab*P + c_p) + gamma

computed in bf16 as three flat element passes over full-W columns:
  1. affine:   b2 = C_ab*colP[wp:wp+FW] + c_p   (DVE 4x tensor_scalar when
               wp is even, else alignment-immune ACT)
  2. STT:      T  = (colQ[wq:wq+FW] + alpha) * b2  (DVE, 2x packed mode)
  3. + gamma   in place on T (DVE 4x tensor_scalar / ACT copy-bias split;
               never GpSimd - its SBUF ops grab the DVE shared port pair
               and stall the DVE packed modes)

Flat full-W columns: every operand is a contiguous FW = BPC*W element
slice of an SBUF column, the per-kernel w-window offset absorbed into
the slice start.  Positions w in [OW, W) per batch item are junk lanes
(the <=2-element overread past a column lands in them / in the 2-element
tile pad); the host slices w < OW after the full-W store.  DVE packed
perf modes need 4-byte-aligned bf16 operand starts, so odd w-offsets are
handled by (a) routing the affine pass to ACT for odd wp and (b) reading
the STT input from a +1-element-preshifted copy of its column.

All relative h-shifts and +1 w-shifts between the windows are resolved
HOST-side into a packed auxiliary DRAM tensor `scol` [H, ncols*FW]
holding exactly the shifted columns used, so on-device loads are two
tensors' worth of full-128-partition, contiguous-per-partition DMAs
(large descriptors spread evenly over the 16 SDMA engines; non-128-row
or small-chunk loads measurably skew/bloat descriptor work).  x channels
are host-permuted into first-use order so early compute groups unblock
after the first load chunk; kernels needing only unshifted columns are
ordered first so compute overlaps the scol load.  Device output layout
[OH, K(sorted), BPC, W] makes each group store one contiguous ~4KB run
per partition; stores issue from the GpSimd queue via SWDGE (async).
Sharding: data-parallel over batch, 2 batch items per core, 8 cores.
"""

import numpy as np

B, C, H, W = 16, 64, 128, 128
K = 128
RH = RW = 3
OH, OW = H - RH + 1, W - RW + 1
NCORES = 8
BPC = B // NCORES
FW = BPC * W  # flat column width (elements per partition per column)

GRP = 8  # kernels per store group
B2_DVE_WHEN_EVEN = True  # pass-1 affine on DVE (4x TS) when its offset is even
GAMMA_DVE_MOD, GAMMA_DVE_LT = 7, 3  # gamma ops with ctr%MOD<LT go to DVE


def _coeffs(weights):
    """Per-kernel coefficients of out = Cab*a*b + Ca*a + Cb*b + C1."""
    w = [weights[:, i].astype(np.float64) for i in range(16)]
    cab = w[1] - w[2] - w[4] - 2 * w[6] - w[7] + w[8] + 2 * w[9] + w[11] + w[13] - w[14]
    ca = w[2] + w[3] + w[6] + w[7] - w[8] - w[9] - w[12] - w[13]
    cb = w[4] + w[5] + w[6] + w[7] - w[8] - w[9] - w[10] - w[11]
    c1 = w[8] + w[9] + w[10] + w[11] + w[12] + w[13] + w[14] + w[15]
    return cab, ca, cb, c1


def _plan(pairs_a, pairs_b, weights):
    """Host-side schedule.

    Returns (plans, layout, order) where plans[k] = (k, base, q_src, p_src,
    path, scal, gamma, b2_dve) with q_src/p_src = (from_scol, column_index,
    w_off); layout = (xperm, scolkeys): xperm = channel order in the device
    x tensor, scolkeys = [(hshift, wshift, chan)] in device scol order;
    order = group-schedulable kernel order."""
    cab, ca, cb, c1 = _coeffs(weights)
    raw = []
    for k in range(K):
        ha, wa, cca = int(pairs_a[k][0]), int(pairs_a[k][1]), int(pairs_a[k][2])
        hb, wb, ccb = int(pairs_b[k][0]), int(pairs_b[k][1]), int(pairs_b[k][2])
        # base row: window row oh lives at partition base+oh; operand side
        # with the larger h needs no h-shift when base = its h.  Choose
        # base = min(ha, hb) so the OTHER side's shift is negative... both
        # work; keep base = hb (P side unshifted) where possible after
        # orientation, else shift is resolved host-side anyway.
        kab, kka, kkb, kk1 = float(cab[k]), float(ca[k]), float(cb[k]), float(c1[k])
        cand = []
        if abs(kab) > 1e-7 and abs(kka * kkb) <= 50.0 * abs(kab):
            if abs(kkb) <= 50.0 * abs(kab):
                cand.append(("B", abs(kkb / kab)))  # P=B, Q=A
            if abs(kka) <= 50.0 * abs(kab):
                cand.append(("A", abs(kka / kab)))  # P=A, Q=B
        if cand:
            # prefer even STT-side offset (DVE 2x); tie-break smaller |alpha|
            def rank(c):
                qoff = wa if c[0] == "B" else wb
                return (qoff % 2, c[1])

            cand.sort(key=rank)
            pside = cand[0][0]
            path = "fact"
            if pside == "B":
                scal = (kab, kka, kkb / kab)
                qh, qw, qc, ph, pw, pc = ha, wa, cca, hb, wb, ccb
            else:
                scal = (kab, kkb, kka / kab)
                qh, qw, qc, ph, pw, pc = hb, wb, ccb, ha, wa, cca
            gamma = kk1 - kka * kkb / kab
        elif abs(kab) <= 1e-7:
            path, scal, gamma = "linear", (kka, kkb, kk1), 0.0
            qh, qw, qc, ph, pw, pc = ha, wa, cca, hb, wb, ccb
        else:
            path, scal, gamma = "exact", (kab, kka, kkb, kk1), 0.0
            qh, qw, qc, ph, pw, pc = ha, wa, cca, hb, wb, ccb
        # q gets a +1 w-preshifted copy when its offset is odd (STT align)
        qsw = qw % 2 if path in ("fact", "linear") else 0
        raw.append(
            (k, qh, qw, qc, qsw, ph, pw, pc, path, scal, gamma)
        )

    # column keys: base = max of the two h's (shifts then non-positive and
    # junk rows stay in the pad range).  key = (hshift, wshift, chan);
    # hshift = h - base <= 0.
    used = {}
    info = []
    for (k, qh, qw, qc, qsw, ph, pw, pc, path, scal, gamma) in raw:
        base = max(qh, ph)
        qkey = (qh - base, qsw, qc)
        pkey = (ph - base, 0, pc)
        for key in (qkey, pkey):
            if key[0] != 0 or key[1] != 0:
                used.setdefault(key, len(used))
        info.append((k, base, qkey, qw - qsw, pkey, pw, path, scal, gamma))

    # schedule order: kernels with both columns unshifted first (they only
    # need the x tensor), then the rest; base-sorted inside for store runs.
    def needs_scol(i):
        _, _, qkey, _, pkey, _, _, _, _ = info[i]
        return (qkey[0], qkey[1]) != (0, 0) or (pkey[0], pkey[1]) != (0, 0)

    order = sorted(range(K), key=lambda i: (needs_scol(i), info[i][1], i))

    # x channel permutation: first-use order over the schedule
    xperm = []
    seen = set()
    for i in order:
        _, _, qkey, _, pkey, _, _, _, _ = info[i]
        for key in (qkey, pkey):
            if key[0] == 0 and key[1] == 0 and key[2] not in seen:
                seen.add(key[2])
                xperm.append(key[2])
    for c in range(C):
        if c not in seen:
            xperm.append(c)
    xpos = {c: j for j, c in enumerate(xperm)}

    # scol keys in first-use order over the schedule
    sperm = []
    sseen = set()
    for i in order:
        _, _, qkey, _, pkey, _, _, _, _ = info[i]
        for key in (qkey, pkey):
            if (key[0] != 0 or key[1] != 0) and key not in sseen:
                sseen.add(key)
                sperm.append(key)
    spos = {key: j for j, key in enumerate(sperm)}

    plans = [None] * K
    for (k, base, qkey, qoff, pkey, poff, path, scal, gamma) in info:
        def src(key, off):
            if key[0] == 0 and key[1] == 0:
                return (False, xpos[key[2]], off)
            return (True, spos[key], off)

        b2_dve = B2_DVE_WHEN_EVEN and path == "fact" and poff % 2 == 0
        plans[k] = (k, base, src(qkey, qoff), src(pkey, poff), path, scal,
                    gamma, b2_dve)

    return plans, (xperm, sperm), order


def _build(pairs_a, pairs_b, weights):
    import concourse.bacc as bacc
    import concourse.mybir as mybir
    from concourse.tile import TileContext

    bf16 = mybir.dt.bfloat16
    Copy = mybir.ActivationFunctionType.Copy
    add, mult = mybir.AluOpType.add, mybir.AluOpType.mult

    plans, (xperm, sperm), order = _plan(pairs_a, pairs_b, weights)
    ncols = max(1, len(sperm))
    ngrp = (K + GRP - 1) // GRP

    nc = bacc.Bacc()
    x = nc.dram_tensor("x", [H, C * FW], bf16, kind="ExternalInput")
    sc = nc.dram_tensor("scol", [H, ncols * FW], bf16, kind="ExternalInput")
    out = nc.dram_tensor("out", [OH, K, BPC, W], bf16, kind="ExternalOutput")

    with TileContext(nc) as tc:
        with (
            tc.tile_pool(name="xp", bufs=1) as xp,
            tc.tile_pool(name="bp", bufs=10) as bp,
            tc.tile_pool(name="tp", bufs=4) as tp,
        ):
            # +2 element pad: flat w-offset views overread <=2 elements past
            # the last column; the pad keeps that read in-bounds.
            X = xp.tile([H, C * FW + 2], bf16)
            nc.gpsimd.memset(X[:, C * FW : C * FW + 2], 0.0)
            S = xp.tile([H, ncols * FW + 2], bf16)
            nc.gpsimd.memset(S[:, ncols * FW : ncols * FW + 2], 0.0)

            # interleaved issue order on one FIFO queue: a small first x
            # chunk unblocks the first compute groups early; scol chunks slot
            # between x chunks (channels are first-use ordered, scol
            # consumers come later).  Competing queues were tried and starve
            # the early groups - shared SDMA engines split bandwidth.
            # tiny leading chunk: compute start = NEFF preamble (~5.7us) +
            # first-chunk transfer + ~2.5us HBM-receipt/sem latency, and
            # chunk 0 (2 first-use-ordered channels) covers kernel 0 exactly
            xbnd = [0, 2 * FW, 10 * FW, 26 * FW, 44 * FW, C * FW]
            sbnd = [0, ncols * FW // 2, ncols * FW]
            seq = [("x", 0), ("x", 1), ("x", 2), ("s", 0), ("x", 3), ("s", 1),
                   ("x", 4)]
            for kind, q in seq:
                if kind == "x":
                    nc.sync.dma_start(
                        out=X[:, xbnd[q] : xbnd[q + 1]],
                        in_=x[:, xbnd[q] : xbnd[q + 1]],
                    )
                else:
                    nc.sync.dma_start(
                        out=S[:, sbnd[q] : sbnd[q + 1]],
                        in_=sc[:, sbnd[q] : sbnd[q + 1]],
                    )

            outv = out.rearrange("oh k b w -> oh (k b w)")

            def col(src, cnt):
                shifted, idx, woff = src
                t = S if shifted else X
                return t[0:cnt, idx * FW + woff : idx * FW + woff + FW]

            gamma_ctr = [0]

            def emit_gamma(j, k, T):
                _, base, _, _, path, scal, gamma, _ = plans[k]
                cnt = base + OH
                slot = T[0:cnt, j * FW : (j + 1) * FW]
                if gamma != 0.0:
                    gamma_ctr[0] += 1
                    if gamma_ctr[0] % GAMMA_DVE_MOD < GAMMA_DVE_LT:
                        nc.vector.tensor_scalar(slot, slot, gamma, None, add)
                    else:
                        nc.scalar.activation(slot, slot, Copy, bias=gamma, scale=1.0)

            def emit_store(g, ks, T, j0=0, j1=None, eng=None):
                # batched stores per same-base run: one contiguous-run DMA per
                # run via SWDGE on the GpSimd queue (async; keeps the Sync
                # FIFO free for loads).  The final flush goes on the by-then
                # idle Sync HWDGE queue instead (faster issue and first byte).
                i = j0
                j1 = len(ks) if j1 is None else j1
                while i < j1:
                    base = plans[ks[i]][1]
                    i2 = i
                    while i2 < j1 and plans[ks[i2]][1] == base:
                        i2 += 1
                    src = T[base : base + OH, i * FW : i2 * FW]
                    dst = outv[0:OH, (g * GRP + i) * FW : (g * GRP + i2) * FW]
                    (eng or nc.gpsimd).dma_start(out=dst, in_=src)
                    i = i2

            def emit_gamma_and_store(g, ks, T):
                # deferred one group so cross-engine waits are pre-satisfied
                for j, k in enumerate(ks):
                    emit_gamma(j, k, T)
                emit_store(g, ks, T)

            pending = None
            for g in range(ngrp):
                ks = order[g * GRP : (g + 1) * GRP]
                last = g == ngrp - 1
                T = tp.tile([H, GRP * FW], bf16, tag="t", name=f"t_{g}")

                for j, k in enumerate(ks):
                    _, base, q_src, p_src, path, scal, gamma, b2_dve = plans[k]
                    cnt = base + OH
                    Qv, Pv = col(q_src, cnt), col(p_src, cnt)
                    slot = T[0:cnt, j * FW : (j + 1) * FW]
                    b2 = bp.tile([H, FW], bf16, tag="b2", name=f"b2_{k}")
                    b2v = b2[0:cnt]

                    if path == "fact":
                        kab, c_p, alpha = scal
                        if b2_dve:
                            nc.vector.tensor_scalar(b2v, Pv, kab, c_p, mult, add)
                        else:
                            nc.scalar.activation(b2v, Pv, Copy, bias=c_p, scale=kab)
                        nc.vector.scalar_tensor_tensor(slot, Qv, alpha, b2v, add, mult)
                    else:  # linear/exact: slot = Ca*Q + (Cb*P + C1)
                        if path == "linear":
                            kka, kkb, kk1 = scal
                        else:
                            kab, kka, kkb, kk1 = scal
                        nc.scalar.activation(b2v, Pv, Copy, bias=kk1, scale=kkb)
                        nc.vector.scalar_tensor_tensor(slot, Qv, kka, b2v, mult, add)
                        if path == "exact":  # += (Cab*P)*Q
                            p2 = bp.tile([H, FW], bf16, tag="b2", name=f"p2_{k}")
                            p2v = p2[0:cnt]
                            nc.vector.scalar_tensor_tensor(p2v, Pv, kab, Qv, mult, mult)
                            nc.vector.tensor_tensor(slot, slot, p2v, add)
                if pending is not None:
                    emit_gamma_and_store(*pending)
                pending = (g, ks, T)
            # final flush: interleave gamma and half-stores so the last
            # store transfer is halved and overlaps the remaining gammas
            g, ks, T = pending
            half = len(ks) // 2
            for j in range(half):
                emit_gamma(j, ks[j], T)
            emit_store(g, ks, T, 0, half)
            for j in range(half, len(ks)):
                emit_gamma(j, ks[j], T)
            emit_store(g, ks, T, half, len(ks))
    nc.compile()
    return nc


def _make(x, pairs_a, pairs_b, weights):
    """Build program + per-core input maps + unshard fn (shared with test)."""
    import ml_dtypes

    x = np.ascontiguousarray(np.asarray(x), dtype=np.float32)
    pa = np.asarray(pairs_a).astype(np.int64)
    pb = np.asarray(pairs_b).astype(np.int64)
    w = np.asarray(weights).astype(np.float32)

    nc = _build(pa, pb, w)
    plans, (xperm, sperm), order = _plan(pa, pb, w)
    ncols = max(1, len(sperm))

    def shards(i):
        # [BPC, C, H, W] -> [H, C(perm), BPC, W] flat, bf16
        xt = (
            x[i * BPC : (i + 1) * BPC]
            .transpose(2, 1, 0, 3)
            .astype(ml_dtypes.bfloat16)
        )  # [H, C, BPC, W]
        xflat = np.ascontiguousarray(xt[:, xperm]).reshape(H, C * FW)
        # scol: host-resolved shifted columns, zero-filled out of range
        scol = np.zeros((H, ncols, FW), ml_dtypes.bfloat16)
        flat = xt.reshape(H, C * FW)
        for j, (sh, sw, c) in enumerate(sperm):
            # scol[p, j, f] = x[p+sh, c*FW + f + sw]
            lo, hi = max(0, -sh), min(H, H - sh)
            seg = flat[lo + sh : hi + sh, c * FW + sw : (c + 1) * FW + sw]
            if seg.shape[1] < FW:  # w-shift ran past the tensor end
                pad = np.zeros((seg.shape[0], FW - seg.shape[1]), ml_dtypes.bfloat16)
                seg = np.concatenate([seg, pad], axis=1)
            scol[lo:hi, j] = seg
        return {"x": xflat, "scol": scol.reshape(H, ncols * FW)}

    in_maps = [shards(i) for i in range(NCORES)]

    pos = np.empty(K, np.int64)
    pos[np.asarray(order)] = np.arange(K)

    def unshard(results):
        # device layout [OH, K(sorted), BPC, W] per core -> [B, K, OH, OW]
        full = np.concatenate(
            [r["out"] for r in results], axis=2
        )  # [OH, K, B, W]
        return np.ascontiguousarray(
            full[:, pos, :, :OW].transpose(2, 1, 0, 3).astype(np.float32)
        )

    return nc, in_maps, unshard


def kernel(x, pairs_a, pairs_b, weights):
    from concourse.bass_utils import run_bass_kernel_spmd

    nc, in_maps, unshard = _make(x, pairs_a, pairs_b, weights)
    res = run_bass_kernel_spmd(nc, in_maps, core_ids=list(range(NCORES)))
    return unshard(res.results)
